# revision 14
# baseline (speedup 1.0000x reference)
"""Trainium2 Bass kernel for nn_ContrastiveLoss (patch-level contrastive loss).

Reference math:
  n1 = normalize(normal_embed)  [N,P,D], n2 = normalize(defect_embed) [M,P,D]
  sim_nn[i,j,q] = max_p <n1[i,p,:], n1[j,q,:]>   (max over first arg's patches)
  sim_nd[i,j,q] = max_p <n1[i,p,:], n2[j,q,:]>
  pos_loss = sum_{i<j,q} (1 - sim_nn[i,j,q]) / (npairs*P)
  neg_loss = mean(relu(sim_nd - 0.5))
  loss = pos_loss + neg_loss

Distribution (8 NeuronCores, data-parallel over i):
  Core c owns moving images I_c = {c, 31-c, 15-c, 16+c} (4 normal images,
  normalized, d-major [768, 4*196]).  The j-side (all 32 normalized normal
  images for nn, all 32 defect for nd; d-major [768, 32*196]) is replicated
  and streamed as 128-wide stationary tiles.  For each stationary q-tile
  (128 q's) the kernel runs a 6-chunk PSUM-accumulated matmul against two
  2-image moving batches (free dim 392), then a free-dim reduce_max gives
  max-over-p per q, collected into SBUF slot matrices.  Final on-device
  stage: masked sum (pos) and relu(x-0.5) sum (neg) + a ones-matmul
  partition reduction -> per-core partial sums [1,2].  Host combines:
  loss = 1 - S_pos/(npairs*P) + S_neg/(N*M*P).
"""

import os

import numpy as np

# Problem constants (hardcoded per the contract; kernel.py is self-contained).
N_IMG = 32
P = 196
D = 768
EPS = 1e-8
MARGIN = 0.5
NCORES = 8
KCHUNKS = D // 128          # 6
Q_ALL = N_IMG * P           # 6272
NT = Q_ALL // 128           # 49 stationary q-tiles per side
TGROUP = 4                  # q-tiles per DMA group (512 q's)
NGROUPS = (NT + TGROUP - 1) // TGROUP   # 13 (last group has 1 tile)
QPAD = NGROUPS * TGROUP * 128           # 6656
NPAIRS = N_IMG * (N_IMG - 1) // 2

# Matmul operand dtype: "f32r" (full-rate fp32 on trn2 PE) or "bf16".
MM_DTYPE = os.environ.get("CL_MM_DTYPE", "f32r")
# Bounce DMA'd matmul operands through a DVE copy so matmuls single-wait on
# the DVE semaphore (1 = on). With 0, Bacc's event-semaphore pass legalizes
# multi-waits instead.
BOUNCE = os.environ.get("CL_BOUNCE", "1") == "1"
# Debug bisection knobs: number of sides (2) and DMA groups per side (13).
DBG_SIDES = int(os.environ.get("CL_SIDES", "2"))
DBG_NG = int(os.environ.get("CL_NG", str(NGROUPS)))

_CACHE = {}


def _iset(c):
    """Moving-image set of core c (balanced for the i<j triangle)."""
    return [c, 31 - c, 15 - c, 16 + c]


def _build_nc(mm_dtype_name):
    import concourse.bacc as bacc
    import concourse.mybir as mybir
    import concourse.tile as tile

    f32 = mybir.dt.float32
    mmdt = {"f32r": mybir.dt.float32r, "bf16": mybir.dt.bfloat16}[mm_dtype_name]

    # Bacc (not plain Bass): its compile() runs move_matmul_waits_to_ldweights
    # + generate_event_semaphores, which legalize multi-semaphore waits for
    # the 1-wait-per-instruction ISA constraint on matmul structs.
    nc = bacc.Bacc("TRN2", target_bir_lowering=False, debug=False)

    mov_d = nc.dram_tensor("mov", [D, 4 * P], mmdt, kind="ExternalInput")
    stat_nn_d = nc.dram_tensor("stat_nn", [D, QPAD], mmdt, kind="ExternalInput")
    stat_nd_d = nc.dram_tensor("stat_nd", [D, QPAD], mmdt, kind="ExternalInput")
    wmask_d = nc.dram_tensor("wmask", [128, 4 * NT], f32, kind="ExternalInput")
    out_d = nc.dram_tensor("out", [1, 2], f32, kind="ExternalOutput")

    with tile.TileContext(nc) as tc:
        with (
            tc.tile_pool(name="const", bufs=1) as const_pool,
            tc.tile_pool(name="movp", bufs=1) as mov_pool,
            tc.tile_pool(name="statp", bufs=3) as stat_pool,
            tc.tile_pool(name="slots", bufs=1) as slot_pool,
            tc.tile_pool(name="psum", bufs=3, space="PSUM") as psum_pool,
            tc.tile_pool(name="psum_f", bufs=1, space="PSUM") as psum_f_pool,
        ):
            # Resident moving operand: [128, chunk, 784].
            # The ISA matmul/ldweights structs fit only ONE sync wait, so every
            # matmul dependency must arrive through a single semaphore. All
            # matmul inputs are therefore bounced DRAM -> bounce tile (DMA) ->
            # operand tile (DVE copy): the PE then only ever waits on the DVE
            # semaphore (merged with the PSUM-recycle dep, which is also DVE).
            mov_sb = mov_pool.tile([128, KCHUNKS, 4 * P], mmdt)
            if BOUNCE:
                mov_bounce = mov_pool.tile([128, KCHUNKS, 4 * P], mmdt)
                nc.sync.dma_start(
                    mov_bounce[:],
                    mov_d[:, :].rearrange("(c k) p -> k c p", k=128),
                )
                nc.vector.tensor_copy(mov_sb[:], mov_bounce[:])
            else:
                nc.sync.dma_start(
                    mov_sb[:],
                    mov_d[:, :].rearrange("(c k) p -> k c p", k=128),
                )

            wmask_sb = const_pool.tile([128, 4 * NT], f32)
            nc.sync.dma_start(wmask_sb[:], wmask_d[:, :])

            ones_sb = const_pool.tile([128, 1], f32)
            nc.vector.memset(ones_sb[:], 1.0)

            # Max-over-p slot matrices: column = 4*t + iloc.
            m_slots = [
                slot_pool.tile(
                    [128, 4 * NT], f32, tag=f"mslots{s}", name=f"mslots{s}"
                )
                for s in range(2)
            ]
            junk = [
                slot_pool.tile([128, 4 * NT], f32, tag=f"junk{s}", name=f"junk{s}")
                for s in range(2)
            ]
            acc2 = const_pool.tile([128, 2], f32)
            nc.vector.memset(m_slots[0][:], 0.0)
            nc.vector.memset(m_slots[1][:], 0.0)

            for side, stat_d in enumerate((stat_nn_d, stat_nd_d)[:DBG_SIDES]):
                for g in range(DBG_NG):
                    n_t = min(TGROUP, NT - g * TGROUP)
                    qw = n_t * 128
                    stat_sb = stat_pool.tile(
                        [128, KCHUNKS, TGROUP * 128], mmdt, tag="stat"
                    )
                    src = stat_d[
                        :, g * TGROUP * 128 : g * TGROUP * 128 + qw
                    ].rearrange("(c k) q -> k c q", k=128)
                    if BOUNCE:
                        stat_bounce = stat_pool.tile(
                            [128, KCHUNKS, TGROUP * 128], mmdt, tag="statb"
                        )
                        nc.sync.dma_start(stat_bounce[:, :, 0:qw], src)
                        nc.vector.tensor_copy(
                            stat_sb[:, :, 0:qw], stat_bounce[:, :, 0:qw]
                        )
                    else:
                        nc.sync.dma_start(stat_sb[:, :, 0:qw], src)
                    for tt in range(n_t):
                        t = g * TGROUP + tt
                        ps_a = psum_pool.tile([128, 2 * P], f32, tag="psA")
                        ps_b = psum_pool.tile([128, 2 * P], f32, tag="psB")
                        for c in range(KCHUNKS):
                            lhsT = stat_sb[:, c, tt * 128 : (tt + 1) * 128]
                            nc.tensor.matmul(
                                ps_a[:],
                                lhsT,
                                mov_sb[:, c, 0 : 2 * P],
                                start=(c == 0),
                                stop=(c == KCHUNKS - 1),
                            )
                            nc.tensor.matmul(
                                ps_b[:],
                                lhsT,
                                mov_sb[:, c, 2 * P : 4 * P],
                                start=(c == 0),
                                stop=(c == KCHUNKS - 1),
                            )
                        # max over p (free dim) for each of the 2 images per bank
                        nc.vector.reduce_max(
                            out=m_slots[side][:, 4 * t : 4 * t + 2],
                            in_=ps_a[:].rearrange("k (i p) -> k i p", p=P),
                            axis=mybir.AxisListType.X,
                        )
                        nc.vector.reduce_max(
                            out=m_slots[side][:, 4 * t + 2 : 4 * t + 4],
                            in_=ps_b[:].rearrange("k (i p) -> k i p", p=P),
                            axis=mybir.AxisListType.X,
                        )

            # pos: acc2[:,0] = sum_q mask * m   (standard ops only — the ANT
            # custom-DVE tensor_tensor_reduce fails at runtime via PJRT/axon)
            nc.vector.tensor_mul(junk[0][:], m_slots[0][:], wmask_sb[:])
            nc.vector.reduce_sum(
                out=acc2[:, 0:1], in_=junk[0][:], axis=mybir.AxisListType.X
            )
            # neg: acc2[:,1] = sum_q relu(m - margin)
            nc.vector.tensor_scalar(
                out=junk[1][:],
                in0=m_slots[1][:],
                scalar1=-MARGIN,
                scalar2=0.0,
                op0=mybir.AluOpType.add,
                op1=mybir.AluOpType.max,
            )
            nc.vector.reduce_sum(
                out=acc2[:, 1:2], in_=junk[1][:], axis=mybir.AxisListType.X
            )
            # partition reduction: [1,2] = ones[128,1].T @ acc2[128,2]
            ps_f = psum_f_pool.tile([1, 2], f32)
            nc.tensor.matmul(ps_f[:], ones_sb[:], acc2[:], start=True, stop=True)
            out_sb = const_pool.tile([1, 2], f32)
            nc.vector.tensor_copy(out_sb[:], ps_f[:])
            nc.sync.dma_start(out_d[:, :], out_sb[:])

    nc.compile()
    return nc


def _np_dtype(mm_dtype_name):
    if mm_dtype_name == "bf16":
        import ml_dtypes

        return ml_dtypes.bfloat16
    return np.float32


def _pack_stat(n, dt):
    """[32,P,D] normalized -> d-major [D, QPAD] (j-major q axis), zero-padded."""
    out = np.zeros((D, QPAD), dtype=dt)
    out[:, :Q_ALL] = n.transpose(2, 0, 1).reshape(D, Q_ALL).astype(dt)
    return np.ascontiguousarray(out)


def _build_in_maps(normal_embed, defect_embed, mm_dtype_name):
    dt = _np_dtype(mm_dtype_name)
    x1 = np.asarray(normal_embed, dtype=np.float32)
    x2 = np.asarray(defect_embed, dtype=np.float32)
    n1 = x1 / (np.sqrt(np.sum(x1 * x1, axis=-1, keepdims=True)) + EPS)
    n2 = x2 / (np.sqrt(np.sum(x2 * x2, axis=-1, keepdims=True)) + EPS)

    stat_nn = _pack_stat(n1, dt)
    stat_nd = _pack_stat(n2, dt)

    in_maps = []
    for c in range(NCORES):
        iset = _iset(c)
        mov = np.ascontiguousarray(
            n1[iset].transpose(2, 0, 1).reshape(D, 4 * P).astype(dt)
        )
        # wmask[qw, 4t+iloc] = 1 iff j(q) > i  with q = 128 t + qw
        q = np.arange(NT * 128)
        jq = q // P  # [NT*128]
        wm = np.zeros((128, 4 * NT), dtype=np.float32)
        for iloc, i_img in enumerate(iset):
            col_mask = (jq > i_img).astype(np.float32).reshape(NT, 128).T  # [128,NT]
            wm[:, iloc::4] = col_mask
        in_maps.append(
            {
                "mov": mov,
                "stat_nn": stat_nn,
                "stat_nd": stat_nd,
                "wmask": np.ascontiguousarray(wm),
            }
        )
    return in_maps


def _get_nc():
    key = ("nc", MM_DTYPE, BOUNCE)
    if key not in _CACHE:
        _CACHE[key] = _build_nc(MM_DTYPE)
    return _CACHE[key]


def _run_on_device(in_maps, trace=False):
    from concourse.bass_utils import run_bass_kernel_spmd

    nc = _get_nc()
    return run_bass_kernel_spmd(
        nc, in_maps, core_ids=list(range(NCORES)), trace=trace
    )


def _combine(results):
    s_pos = 0.0
    s_neg = 0.0
    for r in results:
        o = np.asarray(r["out"], dtype=np.float64)
        s_pos += float(o[0, 0])
        s_neg += float(o[0, 1])
    loss = 1.0 - s_pos / (NPAIRS * P) + s_neg / (N_IMG * N_IMG * P)
    return np.float32(loss)


def kernel(normal_embed, defect_embed):
    in_maps = _build_in_maps(normal_embed, defect_embed, MM_DTYPE)
    res = _run_on_device(in_maps, trace=False)
    return _combine(res.results)
